# revision 19
# baseline (speedup 1.0000x reference)
"""KNN top-16 kernel for Trainium2 (8 NeuronCores, SPMD data-parallel).

Problem: points [4, 8192, 3] fp32 -> nn_idx [4, 8192, 16] int32
(indices of the 16 nearest neighbors by squared L2 distance, jax.lax.top_k
tie semantics: equal values ranked by ascending index).

Strategy:
  - Host: per batch, build a K=24-row bf16 "3-split" factorization of the
    augmented distance matmul   v[i,j] = 2<p_i,p_j> - |p_i|^2 - |p_j|^2
    (= -adj; top-16 of v == 16 nearest neighbors).  Each fp32 operand is
    split into 3 bf16 parts (hi/mid/lo); the 24 K-rows contain all product
    terms down to ~2^-27 relative, ordered small-magnitude-first so the
    per-step fp32 PSUM accumulation reproduces the fp32 reference chain to
    ~1 ulp (measured: 12/524288 top-k index diffs vs the CPU jax reference,
    at the cross-backend noise floor).
  - Device (per core: 4096 query rows x 8192 candidates):
      PE    : [24,128]^T @ [24,512] bf16 matmuls -> PSUM fp32 (v values)
      ACT   : PSUM -> SBUF row buffer copies
      DVE   : per-512-chunk InstMax (top-8) + InstMaxIndex, then a 128-wide
              merge (max / match_replace / max) giving the top-16 values and
              their buffer positions with exact tie semantics
      POOL  : two per-partition local_scatters route the global candidate
              indices to their final rank slots (gather-free index recovery)
  - Sharding: core k handles batch k//2, query rows (k%2)*4096 ... +4096.
    No collectives; full inputs in, full output gathered on host.
"""

import numpy as np
import ml_dtypes
from contextlib import ExitStack

B = 4
N = 8192
K = 16
NQ = 4096          # query rows per core
CH = 1024          # candidate chunk width (top-8 kept per chunk)
NCH = N // CH      # 8 chunks
NBUF = NCH * 8     # 64-wide merge buffer
NBLK = NQ // 128   # 32 blocks of 128 query rows
NEGBIG = -3.0e38

_ORDER = [
    "x_hl", "x_lh", "y_hl", "y_lh", "z_hl", "z_lh",
    "x_mm", "y_mm", "z_mm", "sqA_l", "sqB_l",
    "x_hm", "x_mh", "y_hm", "y_mh", "z_hm", "z_mh", "sqA_m", "sqB_m",
    "x_hh", "y_hh", "z_hh", "sqA_h", "sqB_h",
]


def _split3(v):
    h = v.astype(ml_dtypes.bfloat16).astype(np.float32)
    m = (v - h).astype(ml_dtypes.bfloat16).astype(np.float32)
    l = (v - h - m).astype(ml_dtypes.bfloat16).astype(np.float32)
    return h, m, l


def _build_LR(P):
    """P [N,3] fp32 -> (L [24,N] bf16, R [24,N] bf16) K-row factorization."""
    x, y, z = P[:, 0].copy(), P[:, 1].copy(), P[:, 2].copy()
    sq = (x * x + y * y) + z * z
    ones = np.ones(N, np.float32)
    parts = {}
    for cn, (Lc, Rc) in (("x", (np.float32(2) * x, x)),
                         ("y", (np.float32(2) * y, y)),
                         ("z", (np.float32(2) * z, z))):
        lh, lm, ll = _split3(Lc)
        rh, rm, rl = _split3(Rc)
        parts[f"{cn}_hh"] = (lh, rh)
        parts[f"{cn}_hm"] = (lh, rm)
        parts[f"{cn}_hl"] = (lh, rl)
        parts[f"{cn}_mh"] = (lm, rh)
        parts[f"{cn}_mm"] = (lm, rm)
        parts[f"{cn}_lh"] = (ll, rh)
    ah, am, al = _split3(-sq)
    parts["sqA_h"] = (ah, ones)
    parts["sqA_m"] = (am, ones)
    parts["sqA_l"] = (al, ones)
    parts["sqB_h"] = (ones, ah)
    parts["sqB_m"] = (ones, am)
    parts["sqB_l"] = (ones, al)
    L = np.stack([parts[k][0] for k in _ORDER]).astype(ml_dtypes.bfloat16)
    R = np.stack([parts[k][1] for k in _ORDER]).astype(ml_dtypes.bfloat16)
    return L, R


_cache = {}


def _get_nc():
    if "nc" in _cache:
        return _cache["nc"]

    import concourse.bass as bass
    import concourse.bacc as bacc
    import concourse.mybir as mybir
    import concourse.tile as tile
    from concourse import library_config

    F32 = mybir.dt.float32
    BF16 = mybir.dt.bfloat16
    U16 = mybir.dt.uint16
    I16 = mybir.dt.int16
    I32 = mybir.dt.int32

    nc = bacc.Bacc("TRN2", num_devices=8)

    dL = nc.dram_tensor("L", [24, NQ], BF16, kind="ExternalInput")
    dR = nc.dram_tensor("R", [24, N], BF16, kind="ExternalInput")
    dCB = nc.dram_tensor("CB", [128, NBUF], U16, kind="ExternalInput")
    dTP = nc.dram_tensor("TP", [128, NBUF], I16, kind="ExternalInput")
    dRK = nc.dram_tensor("RK", [128, 16], I16, kind="ExternalInput")
    dOUT = nc.dram_tensor("OUT", [NQ, K], I32, kind="ExternalOutput")

    with tile.TileContext(nc) as tc, ExitStack() as ctx:
        pool = ctx.enter_context(tc.tile_pool(name="pool", bufs=1))
        rowp = ctx.enter_context(tc.tile_pool(name="rowp", bufs=2))
        psum = ctx.enter_context(tc.tile_pool(name="psum", bufs=2, space="PSUM"))
        small = ctx.enter_context(tc.tile_pool(name="small", bufs=3))

        tL = pool.tile([24, NQ], BF16)
        nc.sync.dma_start(tL[:], dL[:])
        tR = pool.tile([24, N], BF16)
        nc.sync.dma_start(tR[:], dR[:])
        tCB = pool.tile([128, NBUF], U16)
        nc.sync.dma_start(tCB[:], dCB[:])
        tTP = pool.tile([128, NBUF], I16)
        nc.sync.dma_start(tTP[:], dTP[:])
        tRK = pool.tile([128, 16], I16)
        nc.sync.dma_start(tRK[:], dRK[:])

        nc.gpsimd.load_library(library_config.local_scatter)

        for blk in range(NBLK):
            q0 = blk * 128
            rowbuf = rowp.tile([128, N], F32, tag="rowbuf", bufs=2)

            # PE: v values into PSUM; ACT: evacuate to SBUF rowbuf
            for half in range(4):
                ps = psum.tile([128, 2048], F32, tag="ps", bufs=2)
                for cc in range(4):
                    c0 = half * 2048 + cc * 512
                    nc.tensor.matmul(
                        ps[:, cc * 512:(cc + 1) * 512],
                        tL[:, q0:q0 + 128],
                        tR[:, c0:c0 + 512],
                        start=True, stop=True,
                    )
                nc.scalar.copy(rowbuf[:, half * 2048:(half + 1) * 2048], ps[:])

            # DVE: per-chunk top-8 values + local indices
            valbuf = small.tile([128, NBUF], F32, tag="valbuf")
            idxbuf = small.tile([128, NBUF], U16, tag="idxbuf")
            for c in range(NCH):
                nc.vector.max(valbuf[:, c * 8:(c + 1) * 8],
                              rowbuf[:, c * CH:(c + 1) * CH])
                nc.vector.max_index(idxbuf[:, c * 8:(c + 1) * 8],
                                    valbuf[:, c * 8:(c + 1) * 8],
                                    rowbuf[:, c * CH:(c + 1) * CH])

            # global candidate index = local + chunk base
            gidx = small.tile([128, NBUF], U16, tag="gidx")
            nc.vector.tensor_tensor(gidx[:], idxbuf[:], tCB[:],
                                    op=mybir.AluOpType.add)

            # merge: top-16 of the 128-entry buffer (tie-exact)
            mm1 = small.tile([128, 8], F32, tag="mm1")
            nc.vector.max(mm1[:], valbuf[:])
            pos = small.tile([128, 16], U16, tag="pos")
            nc.vector.max_index(pos[:, 0:8], mm1[:], valbuf[:])
            vb2 = small.tile([128, NBUF], F32, tag="vb2")
            nc.vector.match_replace(vb2[:], mm1[:], valbuf[:], NEGBIG)
            mm2 = small.tile([128, 8], F32, tag="mm2")
            nc.vector.max(mm2[:], vb2[:])
            nc.vector.max_index(pos[:, 8:16], mm2[:], vb2[:])

            # scatter1: rank_at[p] = 1+rank of buffer slot p (0 elsewhere)
            rank_at = small.tile([128, NBUF], I16, tag="rank_at")
            nc.gpsimd.local_scatter(rank_at[:], tRK[:],
                                    pos[:].bitcast(I16),
                                    channels=128, num_elems=NBUF, num_idxs=16)
            # sidx[p] = rank_at[p]-1 for winners, unique trash slot otherwise
            s_t = small.tile([128, NBUF], I16, tag="s_t")
            nc.gpsimd.tensor_scalar(s_t[:], rank_at[:], 1, None,
                                    op0=mybir.AluOpType.subtract)
            s_m = small.tile([128, NBUF], I16, tag="s_m")
            nc.gpsimd.tensor_scalar(s_m[:], rank_at[:], 0, None,
                                    op0=mybir.AluOpType.is_equal)
            sidx = small.tile([128, NBUF], I16, tag="sidx")
            nc.vector.select(sidx[:], s_m[:], tTP[:], s_t[:])
            # scatter2: out16[rank] = gidx[p]
            out16 = small.tile([128, NBUF + 18], U16, tag="out16")
            nc.gpsimd.local_scatter(out16[:].bitcast(I16),
                                    gidx[:].bitcast(I16), sidx[:],
                                    channels=128, num_elems=NBUF + 18,
                                    num_idxs=NBUF)

            oblk = small.tile([128, K], I32, tag="oblk")
            nc.gpsimd.tensor_copy(oblk[:], out16[:, 0:K])
            nc.sync.dma_start(dOUT[q0:q0 + 128, :], oblk[:])

    nc.compile()
    _cache["nc"] = nc
    return nc


def _consts():
    cb = np.broadcast_to(np.repeat(np.arange(NCH, dtype=np.uint16) * CH, 8),
                         (128, NBUF)).copy()
    tp = np.broadcast_to(np.arange(17, 17 + NBUF, dtype=np.int16),
                         (128, NBUF)).copy()
    rk = np.broadcast_to(np.arange(1, 17, dtype=np.int16), (128, 16)).copy()
    return cb, tp, rk


def _get_runner():
    """AOT-compiled shard_map runner, built once.  The stock
    run_bass_kernel_spmd path re-traces + re-lowers (zstd of the whole BIR)
    on every call (~500ms); this caches the compiled executable and uses
    the bass fast-dispatch (no-effect) path."""
    if "runner" in _cache:
        return _cache["runner"]

    import jax
    import jax.numpy as jnp
    from jax.sharding import Mesh, PartitionSpec
    from jax.experimental.shard_map import shard_map
    from concourse import bass2jax
    import concourse.mybir as mybir

    nc = _get_nc()
    bass2jax.install_neuronx_cc_hook()
    assert nc.dbg_addr is None or not nc.dbg_callbacks
    partition_name = (nc.partition_id_tensor.name
                      if nc.partition_id_tensor is not None else None)

    in_names, out_names, out_avals = [], [], []
    for alloc in nc.m.functions[0].allocations:
        if not isinstance(alloc, mybir.MemoryLocationSet):
            continue
        name = alloc.memorylocations[0].name
        if alloc.kind == "ExternalInput":
            if name != partition_name:
                in_names.append(name)
        elif alloc.kind == "ExternalOutput":
            out_names.append(name)
            out_avals.append(jax.core.ShapedArray(
                tuple(alloc.tensor_shape), mybir.dt.np(alloc.dtype)))

    cb, tp, rk = _consts()
    const_map = {"CB": cb, "TP": tp, "RK": rk}
    bind_in_names = list(in_names) + list(out_names)
    if partition_name is not None:
        bind_in_names.append(partition_name)
    bind_in_names = tuple(bind_in_names)
    n_extra = len(in_names) - 2 + len(out_avals)  # const + zero args

    def _body(L, R, *extra):
        args = []
        it = iter(extra)
        for name in in_names:
            if name == "L":
                args.append(L)
            elif name == "R":
                args.append(R)
            else:
                args.append(next(it))
        for _ in out_avals:
            args.append(next(it))
        if partition_name is not None:
            args.append(bass2jax.partition_id_tensor())
        outs = bass2jax._bass_exec_p.bind(
            *args, out_avals=tuple(out_avals), in_names=bind_in_names,
            out_names=tuple(out_names), lowering_input_output_aliases=(),
            sim_require_finite=True, sim_require_nnan=True, nc=nc)
        return outs[0]

    devices = jax.devices()[:8]
    mesh = Mesh(np.asarray(devices), ("core",))
    P = PartitionSpec
    shard = jax.sharding.NamedSharding(mesh, P("core"))

    # device-resident per-call-invariant operands (consts + output zeros)
    extra_dev = []
    for name in in_names:
        if name in ("L", "R"):
            continue
        c = const_map[name]
        extra_dev.append(jax.device_put(np.tile(c, (8, 1)), shard))
    for av in out_avals:
        z = np.zeros((8 * av.shape[0],) + tuple(av.shape[1:]), av.dtype)
        extra_dev.append(jax.device_put(z, shard))
    assert len(extra_dev) == n_extra

    Lav = jax.ShapeDtypeStruct((8 * 24, NQ), ml_dtypes.bfloat16)
    Rav = jax.ShapeDtypeStruct((8 * 24, N), ml_dtypes.bfloat16)
    extra_avals = [jax.ShapeDtypeStruct(a.shape, a.dtype) for a in extra_dev]

    def _compile():
        f = jax.jit(shard_map(_body, mesh=mesh,
                              in_specs=(P("core"),) * (2 + n_extra),
                              out_specs=P("core"), check_rep=False))
        return f.lower(Lav, Rav, *extra_avals).compile()

    try:
        compiled = bass2jax.fast_dispatch_compile(_compile)
    except Exception:
        compiled = _compile()
    _cache["runner"] = (compiled, extra_dev)
    return _cache["runner"]


def kernel(points: np.ndarray) -> np.ndarray:
    points = np.asarray(points, dtype=np.float32)
    assert points.shape == (B, N, 3), points.shape

    compiled, extra_dev = _get_runner()

    Lg = np.empty((8 * 24, NQ), ml_dtypes.bfloat16)
    Rg = np.empty((8 * 24, N), ml_dtypes.bfloat16)
    for b in range(B):
        L, R = _build_LR(points[b])
        for half in range(2):
            core = 2 * b + half
            Lg[core * 24:(core + 1) * 24] = L[:, half * NQ:(half + 1) * NQ]
            Rg[core * 24:(core + 1) * 24] = R
    out_g = np.asarray(compiled(Lg, Rg, *extra_dev))  # [8*NQ, K]

    out = np.empty((B, N, K), np.int32)
    for core in range(8):
        b, half = core // 2, core % 2
        out[b, half * NQ:(half + 1) * NQ, :] = out_g[core * NQ:(core + 1) * NQ]
    return out



# revision 20
# speedup vs baseline: 1.0108x; 1.0108x over previous
"""KNN top-16 kernel for Trainium2 (8 NeuronCores, SPMD data-parallel).

Problem: points [4, 8192, 3] fp32 -> nn_idx [4, 8192, 16] int32
(indices of the 16 nearest neighbors by squared L2 distance, jax.lax.top_k
tie semantics: equal values ranked by ascending index).

Strategy:
  - Host: per batch, build a K=24-row bf16 "3-split" factorization of the
    augmented distance matmul   v[i,j] = 2<p_i,p_j> - |p_i|^2 - |p_j|^2
    (= -adj; top-16 of v == 16 nearest neighbors).  Each fp32 operand is
    split into 3 bf16 parts (hi/mid/lo); the 24 K-rows contain all product
    terms down to ~2^-27 relative, ordered small-magnitude-first so the
    per-step fp32 PSUM accumulation reproduces the fp32 reference chain to
    ~1 ulp.
  - Device (per core: 4096 query rows x 8192 candidates):
      PE    : [24,128]^T @ [24,512] bf16 matmuls -> PSUM fp32 (v values)
      ACT   : PSUM -> SBUF row buffer copies
      DVE   : per-1024-chunk InstMax (top-8) + InstMaxIndex, then a 64-wide
              merge (max / match_replace / max) giving the top-16 values and
              their buffer positions with exact tie semantics.  1024-wide
              chunks need only 16 full-width DVE scans per block (vs 32 at
              512) and a half-size merge; the coverage risk (>8 of a row's
              true top-16 inside one 1024-chunk) affects 7 of 32768 rows on
              the seed-0 data (~40 extra index mismatches, rel-err ~2e-4).
      POOL  : rank-slot arithmetic (tensor_scalar) and two per-partition
              local_scatters route the global candidate indices to their
              final rank slots (gather-free index recovery).  NOTE: the Pool
              engine only accepts tensor_scalar / tensor_copy / memset /
              ISA-library instructions in walrus codegen — tensor_tensor,
              select and scalar_tensor_tensor are rejected, and gpsimd
              cannot read PSUM.
  - Sharding: core k handles batch k//2, query rows (k%2)*4096 ... +4096.
    No collectives; full inputs in, full output gathered on host.

Engine budget per core (CoreSim): DVE 604us (bottleneck, 2 exact scans of
all candidates), ACT 243us, PE 167us, Pool ~10us; total ~626us.  A
single-scan packed-key variant (quantized value + index in one fp32 key,
sorted by one InstMax pass) simulates at ~355us but is not realizable: the
per-column key construction requires a tensor-tensor op which only DVE can
run (costing exactly the scan it saves), and 16-bit keys lose too much
precision (bf16 ~13k index mismatches, fp16 ~4.3k vs the 2e-2 budget).
"""

import numpy as np
import ml_dtypes
from contextlib import ExitStack

B = 4
N = 8192
K = 16
NQ = 4096          # query rows per core
CH = 1024          # candidate chunk width (top-8 kept per chunk)
NCH = N // CH      # 8 chunks
NBUF = NCH * 8     # 64-wide merge buffer
NBLK = NQ // 128   # 32 blocks of 128 query rows
NEGBIG = -3.0e38

_ORDER = [
    "x_hl", "x_lh", "y_hl", "y_lh", "z_hl", "z_lh",
    "x_mm", "y_mm", "z_mm", "sqA_l", "sqB_l",
    "x_hm", "x_mh", "y_hm", "y_mh", "z_hm", "z_mh", "sqA_m", "sqB_m",
    "x_hh", "y_hh", "z_hh", "sqA_h", "sqB_h",
]


def _split3(v):
    h = v.astype(ml_dtypes.bfloat16).astype(np.float32)
    m = (v - h).astype(ml_dtypes.bfloat16).astype(np.float32)
    l = (v - h - m).astype(ml_dtypes.bfloat16).astype(np.float32)
    return h, m, l


def _build_LR(P):
    """P [N,3] fp32 -> (L [24,N] bf16, R [24,N] bf16) K-row factorization."""
    x, y, z = P[:, 0].copy(), P[:, 1].copy(), P[:, 2].copy()
    sq = (x * x + y * y) + z * z
    ones = np.ones(N, np.float32)
    parts = {}
    for cn, (Lc, Rc) in (("x", (np.float32(2) * x, x)),
                         ("y", (np.float32(2) * y, y)),
                         ("z", (np.float32(2) * z, z))):
        lh, lm, ll = _split3(Lc)
        rh, rm, rl = _split3(Rc)
        parts[f"{cn}_hh"] = (lh, rh)
        parts[f"{cn}_hm"] = (lh, rm)
        parts[f"{cn}_hl"] = (lh, rl)
        parts[f"{cn}_mh"] = (lm, rh)
        parts[f"{cn}_mm"] = (lm, rm)
        parts[f"{cn}_lh"] = (ll, rh)
    ah, am, al = _split3(-sq)
    parts["sqA_h"] = (ah, ones)
    parts["sqA_m"] = (am, ones)
    parts["sqA_l"] = (al, ones)
    parts["sqB_h"] = (ones, ah)
    parts["sqB_m"] = (ones, am)
    parts["sqB_l"] = (ones, al)
    L = np.stack([parts[k][0] for k in _ORDER]).astype(ml_dtypes.bfloat16)
    R = np.stack([parts[k][1] for k in _ORDER]).astype(ml_dtypes.bfloat16)
    return L, R


_cache = {}


def _get_nc():
    if "nc" in _cache:
        return _cache["nc"]

    import concourse.bass as bass
    import concourse.bacc as bacc
    import concourse.mybir as mybir
    import concourse.tile as tile
    from concourse import library_config

    F32 = mybir.dt.float32
    BF16 = mybir.dt.bfloat16
    U16 = mybir.dt.uint16
    I16 = mybir.dt.int16
    I32 = mybir.dt.int32

    nc = bacc.Bacc("TRN2", num_devices=8)

    dL = nc.dram_tensor("L", [24, NQ], BF16, kind="ExternalInput")
    dR = nc.dram_tensor("R", [24, N], BF16, kind="ExternalInput")
    dCB = nc.dram_tensor("CB", [128, NBUF], U16, kind="ExternalInput")
    dTP = nc.dram_tensor("TP", [128, NBUF], I16, kind="ExternalInput")
    dRK = nc.dram_tensor("RK", [128, 16], I16, kind="ExternalInput")
    dOUT = nc.dram_tensor("OUT", [NQ, K], I32, kind="ExternalOutput")

    with tile.TileContext(nc) as tc, ExitStack() as ctx:
        pool = ctx.enter_context(tc.tile_pool(name="pool", bufs=1))
        rowp = ctx.enter_context(tc.tile_pool(name="rowp", bufs=2))
        psum = ctx.enter_context(tc.tile_pool(name="psum", bufs=2, space="PSUM"))
        small = ctx.enter_context(tc.tile_pool(name="small", bufs=3))

        tL = pool.tile([24, NQ], BF16)
        nc.sync.dma_start(tL[:], dL[:])
        tR = pool.tile([24, N], BF16)
        nc.sync.dma_start(tR[:], dR[:])
        tCB = pool.tile([128, NBUF], U16)
        nc.sync.dma_start(tCB[:], dCB[:])
        tTP = pool.tile([128, NBUF], I16)
        nc.sync.dma_start(tTP[:], dTP[:])
        tRK = pool.tile([128, 16], I16)
        nc.sync.dma_start(tRK[:], dRK[:])

        nc.gpsimd.load_library(library_config.local_scatter)

        for blk in range(NBLK):
            q0 = blk * 128
            rowbuf = rowp.tile([128, N], F32, tag="rowbuf", bufs=2)

            # PE: v values into PSUM; ACT: evacuate to SBUF rowbuf
            for half in range(4):
                ps = psum.tile([128, 2048], F32, tag="ps", bufs=2)
                for cc in range(4):
                    c0 = half * 2048 + cc * 512
                    nc.tensor.matmul(
                        ps[:, cc * 512:(cc + 1) * 512],
                        tL[:, q0:q0 + 128],
                        tR[:, c0:c0 + 512],
                        start=True, stop=True,
                    )
                nc.scalar.copy(rowbuf[:, half * 2048:(half + 1) * 2048], ps[:])

            # DVE: per-chunk top-8 values + local indices
            valbuf = small.tile([128, NBUF], F32, tag="valbuf")
            idxbuf = small.tile([128, NBUF], U16, tag="idxbuf")
            for c in range(NCH):
                nc.vector.max(valbuf[:, c * 8:(c + 1) * 8],
                              rowbuf[:, c * CH:(c + 1) * CH])
                nc.vector.max_index(idxbuf[:, c * 8:(c + 1) * 8],
                                    valbuf[:, c * 8:(c + 1) * 8],
                                    rowbuf[:, c * CH:(c + 1) * CH])

            # global candidate index = local + chunk base
            gidx = small.tile([128, NBUF], U16, tag="gidx")
            nc.vector.tensor_tensor(gidx[:], idxbuf[:], tCB[:],
                                    op=mybir.AluOpType.add)

            # merge: top-16 of the 128-entry buffer (tie-exact)
            mm1 = small.tile([128, 8], F32, tag="mm1")
            nc.vector.max(mm1[:], valbuf[:])
            pos = small.tile([128, 16], U16, tag="pos")
            nc.vector.max_index(pos[:, 0:8], mm1[:], valbuf[:])
            vb2 = small.tile([128, NBUF], F32, tag="vb2")
            nc.vector.match_replace(vb2[:], mm1[:], valbuf[:], NEGBIG)
            mm2 = small.tile([128, 8], F32, tag="mm2")
            nc.vector.max(mm2[:], vb2[:])
            nc.vector.max_index(pos[:, 8:16], mm2[:], vb2[:])

            # scatter1: rank_at[p] = 1+rank of buffer slot p (0 elsewhere)
            rank_at = small.tile([128, NBUF], I16, tag="rank_at")
            nc.gpsimd.local_scatter(rank_at[:], tRK[:],
                                    pos[:].bitcast(I16),
                                    channels=128, num_elems=NBUF, num_idxs=16)
            # sidx[p] = rank_at[p]-1 for winners, unique trash slot otherwise
            s_t = small.tile([128, NBUF], I16, tag="s_t")
            nc.gpsimd.tensor_scalar(s_t[:], rank_at[:], 1, None,
                                    op0=mybir.AluOpType.subtract)
            s_m = small.tile([128, NBUF], I16, tag="s_m")
            nc.gpsimd.tensor_scalar(s_m[:], rank_at[:], 0, None,
                                    op0=mybir.AluOpType.is_equal)
            sidx = small.tile([128, NBUF], I16, tag="sidx")
            nc.vector.select(sidx[:], s_m[:], tTP[:], s_t[:])
            # scatter2: out16[rank] = gidx[p]
            out16 = small.tile([128, NBUF + 18], U16, tag="out16")
            nc.gpsimd.local_scatter(out16[:].bitcast(I16),
                                    gidx[:].bitcast(I16), sidx[:],
                                    channels=128, num_elems=NBUF + 18,
                                    num_idxs=NBUF)

            oblk = small.tile([128, K], I32, tag="oblk")
            nc.gpsimd.tensor_copy(oblk[:], out16[:, 0:K])
            nc.sync.dma_start(dOUT[q0:q0 + 128, :], oblk[:])

    nc.compile()
    _cache["nc"] = nc
    return nc


def _consts():
    cb = np.broadcast_to(np.repeat(np.arange(NCH, dtype=np.uint16) * CH, 8),
                         (128, NBUF)).copy()
    tp = np.broadcast_to(np.arange(17, 17 + NBUF, dtype=np.int16),
                         (128, NBUF)).copy()
    rk = np.broadcast_to(np.arange(1, 17, dtype=np.int16), (128, 16)).copy()
    return cb, tp, rk


def _get_runner():
    """AOT-compiled shard_map runner, built once.  The stock
    run_bass_kernel_spmd path re-traces + re-lowers (zstd of the whole BIR)
    on every call (~500ms); this caches the compiled executable and uses
    the bass fast-dispatch (no-effect) path."""
    if "runner" in _cache:
        return _cache["runner"]

    import jax
    import jax.numpy as jnp
    from jax.sharding import Mesh, PartitionSpec
    from jax.experimental.shard_map import shard_map
    from concourse import bass2jax
    import concourse.mybir as mybir

    nc = _get_nc()
    bass2jax.install_neuronx_cc_hook()
    assert nc.dbg_addr is None or not nc.dbg_callbacks
    partition_name = (nc.partition_id_tensor.name
                      if nc.partition_id_tensor is not None else None)

    in_names, out_names, out_avals = [], [], []
    for alloc in nc.m.functions[0].allocations:
        if not isinstance(alloc, mybir.MemoryLocationSet):
            continue
        name = alloc.memorylocations[0].name
        if alloc.kind == "ExternalInput":
            if name != partition_name:
                in_names.append(name)
        elif alloc.kind == "ExternalOutput":
            out_names.append(name)
            out_avals.append(jax.core.ShapedArray(
                tuple(alloc.tensor_shape), mybir.dt.np(alloc.dtype)))

    cb, tp, rk = _consts()
    const_map = {"CB": cb, "TP": tp, "RK": rk}
    bind_in_names = list(in_names) + list(out_names)
    if partition_name is not None:
        bind_in_names.append(partition_name)
    bind_in_names = tuple(bind_in_names)
    n_extra = len(in_names) - 2 + len(out_avals)  # const + zero args

    def _body(L, R, *extra):
        args = []
        it = iter(extra)
        for name in in_names:
            if name == "L":
                args.append(L)
            elif name == "R":
                args.append(R)
            else:
                args.append(next(it))
        for _ in out_avals:
            args.append(next(it))
        if partition_name is not None:
            args.append(bass2jax.partition_id_tensor())
        outs = bass2jax._bass_exec_p.bind(
            *args, out_avals=tuple(out_avals), in_names=bind_in_names,
            out_names=tuple(out_names), lowering_input_output_aliases=(),
            sim_require_finite=True, sim_require_nnan=True, nc=nc)
        return outs[0]

    devices = jax.devices()[:8]
    mesh = Mesh(np.asarray(devices), ("core",))
    P = PartitionSpec
    shard = jax.sharding.NamedSharding(mesh, P("core"))

    # device-resident per-call-invariant operands (consts + output zeros)
    extra_dev = []
    for name in in_names:
        if name in ("L", "R"):
            continue
        c = const_map[name]
        extra_dev.append(jax.device_put(np.tile(c, (8, 1)), shard))
    for av in out_avals:
        z = np.zeros((8 * av.shape[0],) + tuple(av.shape[1:]), av.dtype)
        extra_dev.append(jax.device_put(z, shard))
    assert len(extra_dev) == n_extra

    Lav = jax.ShapeDtypeStruct((8 * 24, NQ), ml_dtypes.bfloat16)
    Rav = jax.ShapeDtypeStruct((8 * 24, N), ml_dtypes.bfloat16)
    extra_avals = [jax.ShapeDtypeStruct(a.shape, a.dtype) for a in extra_dev]

    def _compile():
        f = jax.jit(shard_map(_body, mesh=mesh,
                              in_specs=(P("core"),) * (2 + n_extra),
                              out_specs=P("core"), check_rep=False))
        return f.lower(Lav, Rav, *extra_avals).compile()

    try:
        compiled = bass2jax.fast_dispatch_compile(_compile)
    except Exception:
        compiled = _compile()
    _cache["runner"] = (compiled, extra_dev)
    return _cache["runner"]


def kernel(points: np.ndarray) -> np.ndarray:
    points = np.asarray(points, dtype=np.float32)
    assert points.shape == (B, N, 3), points.shape

    compiled, extra_dev = _get_runner()

    Lg = np.empty((8 * 24, NQ), ml_dtypes.bfloat16)
    Rg = np.empty((8 * 24, N), ml_dtypes.bfloat16)
    for b in range(B):
        L, R = _build_LR(points[b])
        for half in range(2):
            core = 2 * b + half
            Lg[core * 24:(core + 1) * 24] = L[:, half * NQ:(half + 1) * NQ]
            Rg[core * 24:(core + 1) * 24] = R
    out_g = np.asarray(compiled(Lg, Rg, *extra_dev))  # [8*NQ, K]

    out = np.empty((B, N, K), np.int32)
    for core in range(8):
        b, half = core // 2, core % 2
        out[b, half * NQ:(half + 1) * NQ, :] = out_g[core * NQ:(core + 1) * NQ]
    return out

